# revision 26
# baseline (speedup 1.0000x reference)
"""Int8Linear TRN2 kernel: y = x @ (W_int8 * scale)^T + bias.

Column-parallel across 8 NeuronCores: each core owns a [2048, 4096] shard
of W, the full x, and its bias slice.

Device strategy per core (v8):
  - ALL weight traffic rides SWDGE (gpsimd DMA, ~410 GB/s measured) except
    one early chunk primed on the otherwise-idle HWDGE ring
  - per 128-row k-chunk, the weights arrive as one of:
      F: casting DMA straight to fp8e4 (1 B/weight, no engine work; the
         TRN fp8 rounding on ~1/3 of the contraction adds ~1e-2 rms,
         within the 2e-2 gate - verified in sim + HW)
      S: casting DMA to bf16 (2 B/weight, no engine work)
      D/A: raw int8 + DVE (~1.2us) / ACT (~2.0us) dequant to bf16
  - PE uses 4-way column tiling (tile_position=(0, 32j)): four concurrent
    [128k, 16m] x [128k, 512o] matmul streams accumulate into one PSUM bank
  - x*scale is bf16 (hi only)
  - bias folds in as a final accumulation matmul against a one-hot lhsT;
    the epilogue is PSUM->SBUF copies on DVE and ACT in parallel, each
    chased by its own output DMA
"""

import os

import numpy as np

IN_F = 4096
OUT_F = 16384
NT = 16
NCORES = 8
O_PER = OUT_F // NCORES  # 2048
NCH = IN_F // 128  # 32 k-chunks
NTJ = 4  # PE column tiles
NMM = 512  # moving free size per matmul

# Per-chunk source assignment (see module docstring). F heads the stream
# (PE starts without engine casts) and tails it (no cast backlog at the
# end); S relieves the engines mid-stream.
ASSIGN = [
    "F", "F", "F", "D", "A", "F", "D", "A", "D", "D", "S", "A", "D", "D", "A", "D",
    "F", "F", "D", "A", "D", "D", "S", "A", "D", "D", "A", "D", "F", "F", "F", "F",
]
assert len(ASSIGN) == NCH

_CACHE = {}
LAST_EXEC_NS = None


def _install_drain_patch():
    """walrus codegen only allows 1 sem-wait per SP instruction; Tile's
    kernel-tail Drain aggregates many. Split them across sync nops."""
    from concourse.tile import TileContext
    from concourse.tile_scheduler import N_PROCS
    from concourse.vector_clock import VectorClock
    from bass_rust import ScopedClock

    if getattr(TileContext, "_drain_patched", False):
        return

    def _patched(self, tick_clock, wait_clock):
        gc = tick_clock.global_clock
        ticks = [gc[p] for p in range(N_PROCS)]
        for i in range(N_PROCS):
            partial = VectorClock(
                [ticks[p] if p == i else 0 for p in range(N_PROCS)]
            )
            if all(t == 0 for t in partial):
                continue
            nop = self.nc.sync.nop(hint="tail_wait", nofuse=True)
            wait_clock.add_sem_waits(nop.ins, ScopedClock({None: partial}))
        self.nc.sync.drain()
        self.nc.all_engine_barrier()
        assert self.sems is not None
        popped = self.nc._tile_sem_poison_stack.pop()
        assert popped is self._sem_poison
        self.nc.clear_and_free_semaphores(list(self.sems.allocated().values()))
        self.nc.all_engine_barrier()

    TileContext._drain_and_barrier = _patched
    TileContext._drain_patched = True


def _split_multi_waits(nc):
    """walrus codegen allows only one sem-wait per instruction: hoist all
    but the last wait of any instruction onto same-engine NoOps before it."""
    from concourse import mybir

    cnt = 0
    for fn in nc.m.functions:
        for bb in fn.blocks:
            out = []
            for inst in bb.instructions:
                si = inst.sync_info
                if si is not None and si.on_wait and len(si.on_wait) > 1:
                    waits = list(si.on_wait)
                    for w in waits[:-1]:
                        cnt += 1
                        nop = mybir.InstNoOp(
                            name=f"{inst.name}-sw{cnt}", ins=[], outs=[]
                        )
                        nop.engine = inst.engine
                        nop.sync_info = mybir.SyncInfo(on_wait=[w], on_update=[])
                        out.append(nop)
                    si.on_wait = [waits[-1]]
                out.append(inst)
            bb.instructions[:] = out


def _build_nc(for_hw=True):
    import concourse.bass as bass
    import concourse.mybir as mybir
    from concourse.tile import TileContext

    _install_drain_patch()

    nc = bass.Bass(trn_type="TRN2")
    # xt cols [16n, 16n+16) = k-chunk n of (x*scale)^T in bf16;
    # cols [512:528) = one-hot block (partition 0 = 1.0) for the bias matmul
    xt = nc.dram_tensor("xt", [128, NCH * NT + NT], mybir.dt.bfloat16, kind="ExternalInput")
    # wt col = n*2048 + j*512 + c  =  W[o = j*512 + c, k = n*128 + p]
    wt = nc.dram_tensor("wt", [128, IN_F * O_PER // 128], mybir.dt.int8, kind="ExternalInput")
    bs = nc.dram_tensor("bs", [1, O_PER], mybir.dt.bfloat16, kind="ExternalInput")
    y = nc.dram_tensor("y", [NT, O_PER], mybir.dt.float32, kind="ExternalOutput")

    with TileContext(nc) as tc:
        with (
            tc.tile_pool(name="xp", bufs=1) as xp,
            tc.tile_pool(name="bp", bufs=1) as bp,
            tc.tile_pool(name="w8", bufs=6) as w8p,
            tc.tile_pool(name="wb", bufs=12) as wbp,
            tc.tile_pool(name="ws", bufs=2) as wsp,
            tc.tile_pool(name="wf", bufs=4) as wfp,
            tc.tile_pool(name="pp", bufs=1, space="PSUM") as pp,
            tc.tile_pool(name="op", bufs=1) as op,
        ):
            # xt rides the fast SWDGE queue ahead of the weight stream;
            # the bias plane (row 0 = bias bf16, rest zeroed by ACT while it
            # is otherwise idle) feeds the tail bias-accumulation matmul
            xsb = xp.tile([128, NCH * NT + NT], mybir.dt.bfloat16)
            nc.gpsimd.dma_start(out=xsb[:], in_=xt[:])
            bsb = bp.tile([128, O_PER], mybir.dt.bfloat16)
            nc.scalar.memzero(bsb[:])
            nc.sync.dma_start(out=bsb[0:1, :], in_=bs[:])
            ones = xsb[:, NCH * NT : NCH * NT + NT]  # [128, 16] one-hot

            psum = pp.tile([128, NMM], mybir.dt.float32, tag="ps", name="ps")

            # plan DMA ops: adjacent same-kind chunks merge into [128, 4096]
            units = []  # (kind, [n...])
            n = 0
            while n < NCH:
                kind = ASSIGN[n] if ASSIGN[n] in ("S", "F") else "R"
                nkind = (
                    (ASSIGN[n + 1] if ASSIGN[n + 1] in ("S", "F") else "R")
                    if n + 1 < NCH
                    else None
                )
                if kind in ("R", "F") and nkind == kind and n != 3:
                    units.append((kind, [n, n + 1]))
                    n += 2
                else:
                    units.append((kind, [n]))
                    n += 1

            wsrc = {}
            for kind, ns in units:
                base = ns[0] * 2048
                span = 2048 * len(ns)
                if kind == "F":
                    tag = "wf" if len(ns) == 2 else "wf1"
                    wb = wfp.tile(
                        [128, span], mybir.dt.float8e4, tag=tag, name=f"wf_{ns[0]}"
                    )
                    nc.gpsimd.dma_start(out=wb[:], in_=wt[:, base : base + span])
                    for k, nn in enumerate(ns):
                        wsrc[nn] = (wb, k * 2048)
                elif kind == "S":
                    wb = wsp.tile(
                        [128, span], mybir.dt.bfloat16, tag="wbs", name=f"ws_{ns[0]}"
                    )
                    nc.gpsimd.dma_start(out=wb[:], in_=wt[:, base : base + span])
                    wsrc[ns[0]] = (wb, 0)
                else:
                    tag = "w8" if len(ns) == 2 else "w81"
                    w8 = w8p.tile(
                        [128, span], mybir.dt.int8, tag=tag, name=f"w8_{ns[0]}"
                    )
                    if ns[0] == 3:
                        # prime the cast engines ~2us early via the
                        # otherwise-idle HWDGE ring
                        nc.sync.dma_start(out=w8[:], in_=wt[:, base : base + span])
                    else:
                        nc.gpsimd.dma_start(out=w8[:], in_=wt[:, base : base + span])
                    for k, nn in enumerate(ns):
                        wb = wbp.tile(
                            [128, 2048], mybir.dt.bfloat16, tag="wb", name=f"wb_{nn}"
                        )
                        src = w8[:, k * 2048 : (k + 1) * 2048]
                        if ASSIGN[nn] == "A":
                            nc.scalar.copy(wb[:], src)
                        else:
                            nc.vector.tensor_copy(wb[:], src)
                        wsrc[nn] = (wb, 0)

            # matmul stream: 4 concurrent column-tile chains over 32 chunks
            for n in range(NCH):
                wb, off = wsrc[n]
                for j in range(NTJ):
                    nc.tensor.matmul(
                        psum[32 * j : 32 * j + NT, :],
                        lhsT=xsb[:, NT * n : NT * (n + 1)],
                        rhs=wb[:, off + j * 512 : off + (j + 1) * 512],
                        start=(n == 0),
                        stop=False,
                        tile_position=(0, 32 * j),
                        skip_group_check=True,
                    )
            # bias folds in as the final accumulation against a one-hot
            # lhsT; per column group: bias matmul -> copy -> output DMA so
            # the four groups pipeline across PE, DVE/ACT and the DMA ring
            osb = op.tile([NT, O_PER], mybir.dt.float32, tag="o")
            for j in range(NTJ):
                nc.tensor.matmul(
                    psum[32 * j : 32 * j + NT, :],
                    lhsT=ones,
                    rhs=bsb[:, j * 512 : (j + 1) * 512],
                    start=False,
                    stop=True,
                    tile_position=(0, 32 * j),
                    skip_group_check=True,
                )
            for j in range(NTJ):
                dst = osb[:, j * 512 : (j + 1) * 512]
                srcp = psum[32 * j : 32 * j + NT, :]
                if j % 2 == 0:
                    nc.vector.tensor_copy(dst, srcp)
                else:
                    nc.scalar.copy(dst, srcp)
                nc.sync.dma_start(
                    out=y[:, j * 512 : (j + 1) * 512],
                    in_=dst,
                )
    if for_hw:
        _split_multi_waits(nc)
    return nc


def _prep(x, w, scale, bias):
    import ml_dtypes

    xs = (np.asarray(x, dtype=np.float32) * np.float32(scale)).astype(
        ml_dtypes.bfloat16
    )
    # xt[p, 16n + t] = xs[t, n*128 + p]; one-hot block for the bias matmul
    xt_host = np.zeros((128, NCH * NT + NT), dtype=ml_dtypes.bfloat16)
    xt_host[:, : NCH * NT] = (
        xs.T.reshape(NCH, 128, NT).transpose(1, 0, 2).reshape(128, NCH * NT)
    )
    xt_host[0, NCH * NT :] = ml_dtypes.bfloat16(1.0)
    return xt_host


def _prep_w(wshard):
    # wt2[p, n*2048 + j*512 + c] = wshard[j*512 + c, n*128 + p]
    A = np.ascontiguousarray(wshard.T)  # [4096 k, 2048 o]
    A4 = A.reshape(NCH, 128, O_PER)  # (n, p, o)
    return np.ascontiguousarray(
        A4.transpose(1, 0, 2).reshape(128, IN_F * O_PER // 128)
    )


def kernel(x, weight_int8, weight_scale, bias):
    global LAST_EXEC_NS
    import ml_dtypes
    from concourse.bass_utils import run_bass_kernel_spmd

    w = np.asarray(weight_int8)
    if w.dtype != np.int8:
        w = w.astype(np.int8)
    scale = float(np.asarray(weight_scale, dtype=np.float32))
    bias = np.asarray(bias, dtype=np.float32)

    xt_host = _prep(x, w, scale, bias)

    if "nc" not in _CACHE:
        _CACHE["nc"] = _build_nc()
    nc = _CACHE["nc"]

    in_maps = []
    for c in range(NCORES):
        wt_c = _prep_w(w[c * O_PER : (c + 1) * O_PER, :])
        bs_c = np.ascontiguousarray(
            bias[c * O_PER : (c + 1) * O_PER].astype(ml_dtypes.bfloat16)[None, :]
        )
        in_maps.append({"xt": xt_host, "wt": wt_c, "bs": bs_c})

    trace = bool(os.environ.get("BASS_KERNEL_TRACE"))
    br = run_bass_kernel_spmd(nc, in_maps, list(range(NCORES)), trace=trace)
    LAST_EXEC_NS = br.exec_time_ns
    return np.concatenate([br.results[c]["y"] for c in range(NCORES)], axis=1)


# revision 31
# speedup vs baseline: 1.0333x; 1.0333x over previous
"""Int8Linear TRN2 kernel: y = x @ (W_int8 * scale)^T + bias.

Column-parallel across 8 NeuronCores: each core owns a [2048, 4096] shard
of W, the full x, and its bias slice.

Device strategy per core (v8):
  - ALL weight traffic rides SWDGE (gpsimd DMA, ~410 GB/s measured) except
    one early chunk primed on the otherwise-idle HWDGE ring
  - per 128-row k-chunk, the weights arrive as one of:
      F: casting DMA straight to fp8e4 (1 B/weight, no engine work; the
         TRN fp8 rounding on ~1/3 of the contraction adds ~1e-2 rms,
         within the 2e-2 gate - verified in sim + HW)
      S: casting DMA to bf16 (2 B/weight, no engine work)
      D/A: raw int8 + DVE (~1.2us) / ACT (~2.0us) dequant to bf16
  - PE uses 4-way column tiling (tile_position=(0, 32j)): four concurrent
    [128k, 16m] x [128k, 512o] matmul streams accumulate into one PSUM bank
  - x*scale is bf16 (hi only)
  - bias folds in as a final accumulation matmul against a one-hot lhsT;
    the epilogue is PSUM->SBUF copies on DVE and ACT in parallel, each
    chased by its own output DMA
"""

import os

import numpy as np

IN_F = 4096
OUT_F = 16384
NT = 16
NCORES = 8
O_PER = OUT_F // NCORES  # 2048
NCH = IN_F // 128  # 32 k-chunks
NTJ = 4  # PE column tiles
NMM = 512  # moving free size per matmul

# Per-chunk source assignment (see module docstring). F heads the stream
# (PE starts without engine casts) and tails it (no cast backlog at the
# end); S relieves the engines mid-stream.
ASSIGN = [
    "F", "F", "D", "A", "D", "S", "D", "A", "F", "D", "D", "A", "D", "S", "D", "A",
    "F", "F", "D", "A", "D", "S", "D", "A", "F", "D", "D", "S", "D", "D", "F", "F",
]
assert len(ASSIGN) == NCH

_CACHE = {}
LAST_EXEC_NS = None


def _install_drain_patch():
    """walrus codegen only allows 1 sem-wait per SP instruction; Tile's
    kernel-tail Drain aggregates many. Split them across sync nops."""
    from concourse.tile import TileContext
    from concourse.tile_scheduler import N_PROCS
    from concourse.vector_clock import VectorClock
    from bass_rust import ScopedClock

    if getattr(TileContext, "_drain_patched", False):
        return

    def _patched(self, tick_clock, wait_clock):
        gc = tick_clock.global_clock
        ticks = [gc[p] for p in range(N_PROCS)]
        for i in range(N_PROCS):
            partial = VectorClock(
                [ticks[p] if p == i else 0 for p in range(N_PROCS)]
            )
            if all(t == 0 for t in partial):
                continue
            nop = self.nc.sync.nop(hint="tail_wait", nofuse=True)
            wait_clock.add_sem_waits(nop.ins, ScopedClock({None: partial}))
        self.nc.sync.drain()
        self.nc.all_engine_barrier()
        assert self.sems is not None
        popped = self.nc._tile_sem_poison_stack.pop()
        assert popped is self._sem_poison
        self.nc.clear_and_free_semaphores(list(self.sems.allocated().values()))
        self.nc.all_engine_barrier()

    TileContext._drain_and_barrier = _patched
    TileContext._drain_patched = True


def _split_multi_waits(nc):
    """walrus codegen allows only one sem-wait per instruction: hoist all
    but the last wait of any instruction onto same-engine NoOps before it."""
    from concourse import mybir

    cnt = 0
    for fn in nc.m.functions:
        for bb in fn.blocks:
            out = []
            for inst in bb.instructions:
                si = inst.sync_info
                if si is not None and si.on_wait and len(si.on_wait) > 1:
                    waits = list(si.on_wait)
                    for w in waits[:-1]:
                        cnt += 1
                        nop = mybir.InstNoOp(
                            name=f"{inst.name}-sw{cnt}", ins=[], outs=[]
                        )
                        nop.engine = inst.engine
                        nop.sync_info = mybir.SyncInfo(on_wait=[w], on_update=[])
                        out.append(nop)
                    si.on_wait = [waits[-1]]
                out.append(inst)
            bb.instructions[:] = out


def _build_nc(for_hw=True):
    import concourse.bass as bass
    import concourse.mybir as mybir
    from concourse.tile import TileContext

    _install_drain_patch()

    nc = bass.Bass(trn_type="TRN2")
    # xt cols [16n, 16n+16) = k-chunk n of (x*scale)^T in bf16;
    # cols [512:528) = one-hot block (partition 0 = 1.0) for the bias matmul
    xt = nc.dram_tensor("xt", [128, NCH * NT + NT], mybir.dt.bfloat16, kind="ExternalInput")
    # wt col = n*2048 + j*512 + c  =  W[o = j*512 + c, k = n*128 + p]
    wt = nc.dram_tensor("wt", [128, IN_F * O_PER // 128], mybir.dt.int8, kind="ExternalInput")
    bs = nc.dram_tensor("bs", [1, O_PER], mybir.dt.bfloat16, kind="ExternalInput")
    y = nc.dram_tensor("y", [NT, O_PER], mybir.dt.float32, kind="ExternalOutput")

    with TileContext(nc) as tc:
        with (
            tc.tile_pool(name="xp", bufs=1) as xp,
            tc.tile_pool(name="bp", bufs=1) as bp,
            tc.tile_pool(name="w8", bufs=6) as w8p,
            tc.tile_pool(name="wb", bufs=12) as wbp,
            tc.tile_pool(name="ws", bufs=4) as wsp,
            tc.tile_pool(name="wf", bufs=4) as wfp,
            tc.tile_pool(name="pp", bufs=1, space="PSUM") as pp,
            tc.tile_pool(name="op", bufs=1) as op,
        ):
            # xt rides the fast SWDGE queue ahead of the weight stream;
            # the bias plane (row 0 = bias bf16, rest zeroed by ACT while it
            # is otherwise idle) feeds the tail bias-accumulation matmul
            xsb = xp.tile([128, NCH * NT + NT], mybir.dt.bfloat16)
            nc.gpsimd.dma_start(out=xsb[:], in_=xt[:])
            bsb = bp.tile([128, O_PER], mybir.dt.bfloat16)
            nc.scalar.memzero(bsb[:])
            nc.sync.dma_start(out=bsb[0:1, :], in_=bs[:])
            ones = xsb[:, NCH * NT : NCH * NT + NT]  # [128, 16] one-hot

            psum = pp.tile([128, NMM], mybir.dt.float32, tag="ps", name="ps")

            # PE warmup: the HAM clock gate only lifts to 2.4 GHz after
            # ~3.4us of sustained matmul activity; burn that in on a scratch
            # PSUM bank while the DMA pipe fills, so the real stream runs
            # warm (cold N=512 matmuls measured 739ns vs ~230ns warm)
            wup = pp.tile([128, NMM], mybir.dt.float32, tag="wu", name="wu")
            for wi in range(14):
                nc.tensor.matmul(
                    wup[0:NT, :],
                    lhsT=ones,
                    rhs=xsb[:, 0:NMM],
                    start=(wi == 0),
                    stop=(wi == 13),
                    tile_position=(0, 0),
                    skip_group_check=True,
                )

            # plan DMA ops: adjacent same-kind chunks merge into [128, 4096]
            units = []  # (kind, [n...])
            n = 0
            while n < NCH:
                kind = ASSIGN[n] if ASSIGN[n] in ("S", "F") else "R"
                nkind = (
                    (ASSIGN[n + 1] if ASSIGN[n + 1] in ("S", "F") else "R")
                    if n + 1 < NCH
                    else None
                )
                if kind in ("R", "F") and nkind == kind and n != 2:
                    units.append((kind, [n, n + 1]))
                    n += 2
                else:
                    units.append((kind, [n]))
                    n += 1

            wsrc = {}
            for kind, ns in units:
                base = ns[0] * 2048
                span = 2048 * len(ns)
                if kind == "F":
                    tag = "wf" if len(ns) == 2 else "wf1"
                    wb = wfp.tile(
                        [128, span], mybir.dt.float8e4, tag=tag, name=f"wf_{ns[0]}"
                    )
                    nc.gpsimd.dma_start(out=wb[:], in_=wt[:, base : base + span])
                    for k, nn in enumerate(ns):
                        wsrc[nn] = (wb, k * 2048)
                elif kind == "S":
                    wb = wsp.tile(
                        [128, span], mybir.dt.bfloat16, tag="wbs", name=f"ws_{ns[0]}"
                    )
                    nc.gpsimd.dma_start(out=wb[:], in_=wt[:, base : base + span])
                    wsrc[ns[0]] = (wb, 0)
                else:
                    tag = "w8" if len(ns) == 2 else "w81"
                    w8 = w8p.tile(
                        [128, span], mybir.dt.int8, tag=tag, name=f"w8_{ns[0]}"
                    )
                    if ns[0] == 2:
                        # prime the cast engines ~2us early via the
                        # otherwise-idle HWDGE ring
                        nc.sync.dma_start(out=w8[:], in_=wt[:, base : base + span])
                    else:
                        nc.gpsimd.dma_start(out=w8[:], in_=wt[:, base : base + span])
                    for k, nn in enumerate(ns):
                        wb = wbp.tile(
                            [128, 2048], mybir.dt.bfloat16, tag="wb", name=f"wb_{nn}"
                        )
                        src = w8[:, k * 2048 : (k + 1) * 2048]
                        if ASSIGN[nn] == "A":
                            nc.scalar.copy(wb[:], src)
                        else:
                            nc.vector.tensor_copy(wb[:], src)
                        wsrc[nn] = (wb, 0)

            # matmul stream: 4 concurrent column-tile chains over 32 chunks
            for n in range(NCH):
                wb, off = wsrc[n]
                for j in range(NTJ):
                    nc.tensor.matmul(
                        psum[32 * j : 32 * j + NT, :],
                        lhsT=xsb[:, NT * n : NT * (n + 1)],
                        rhs=wb[:, off + j * 512 : off + (j + 1) * 512],
                        start=(n == 0),
                        stop=False,
                        tile_position=(0, 32 * j),
                        skip_group_check=True,
                    )
            # bias folds in as the final accumulation against a one-hot
            # lhsT; per column group: bias matmul -> copy -> output DMA so
            # the four groups pipeline across PE, DVE/ACT and the DMA ring
            osb = op.tile([NT, O_PER], mybir.dt.float32, tag="o")
            for j in range(NTJ):
                nc.tensor.matmul(
                    psum[32 * j : 32 * j + NT, :],
                    lhsT=ones,
                    rhs=bsb[:, j * 512 : (j + 1) * 512],
                    start=False,
                    stop=True,
                    tile_position=(0, 32 * j),
                    skip_group_check=True,
                )
            for j in range(NTJ):
                dst = osb[:, j * 512 : (j + 1) * 512]
                srcp = psum[32 * j : 32 * j + NT, :]
                if j % 2 == 0:
                    nc.vector.tensor_copy(dst, srcp)
                else:
                    nc.scalar.copy(dst, srcp)
                nc.sync.dma_start(
                    out=y[:, j * 512 : (j + 1) * 512],
                    in_=dst,
                )
    if for_hw:
        _split_multi_waits(nc)
    return nc


def _prep(x, w, scale, bias):
    import ml_dtypes

    xs = (np.asarray(x, dtype=np.float32) * np.float32(scale)).astype(
        ml_dtypes.bfloat16
    )
    # xt[p, 16n + t] = xs[t, n*128 + p]; one-hot block for the bias matmul
    xt_host = np.zeros((128, NCH * NT + NT), dtype=ml_dtypes.bfloat16)
    xt_host[:, : NCH * NT] = (
        xs.T.reshape(NCH, 128, NT).transpose(1, 0, 2).reshape(128, NCH * NT)
    )
    xt_host[0, NCH * NT :] = ml_dtypes.bfloat16(1.0)
    return xt_host


def _prep_w(wshard):
    # wt2[p, n*2048 + j*512 + c] = wshard[j*512 + c, n*128 + p]
    A = np.ascontiguousarray(wshard.T)  # [4096 k, 2048 o]
    A4 = A.reshape(NCH, 128, O_PER)  # (n, p, o)
    return np.ascontiguousarray(
        A4.transpose(1, 0, 2).reshape(128, IN_F * O_PER // 128)
    )


def kernel(x, weight_int8, weight_scale, bias):
    global LAST_EXEC_NS
    import ml_dtypes
    from concourse.bass_utils import run_bass_kernel_spmd

    w = np.asarray(weight_int8)
    if w.dtype != np.int8:
        w = w.astype(np.int8)
    scale = float(np.asarray(weight_scale, dtype=np.float32))
    bias = np.asarray(bias, dtype=np.float32)

    xt_host = _prep(x, w, scale, bias)

    if "nc" not in _CACHE:
        _CACHE["nc"] = _build_nc()
    nc = _CACHE["nc"]

    in_maps = []
    for c in range(NCORES):
        wt_c = _prep_w(w[c * O_PER : (c + 1) * O_PER, :])
        bs_c = np.ascontiguousarray(
            bias[c * O_PER : (c + 1) * O_PER].astype(ml_dtypes.bfloat16)[None, :]
        )
        in_maps.append({"xt": xt_host, "wt": wt_c, "bs": bs_c})

    trace = bool(os.environ.get("BASS_KERNEL_TRACE"))
    br = run_bass_kernel_spmd(nc, in_maps, list(range(NCORES)), trace=trace)
    LAST_EXEC_NS = br.exec_time_ns
    return np.concatenate([br.results[c]["y"] for c in range(NCORES)], axis=1)


# revision 33
# speedup vs baseline: 1.0422x; 1.0086x over previous
"""Int8Linear TRN2 kernel: y = x @ (W_int8 * scale)^T + bias.

Column-parallel across 8 NeuronCores: each core owns a [2048, 4096] shard
of W, the full x, and its bias slice.

Device strategy per core (v8):
  - ALL weight traffic rides SWDGE (gpsimd DMA, ~410 GB/s measured) except
    one early chunk primed on the otherwise-idle HWDGE ring
  - per 128-row k-chunk, the weights arrive as one of:
      F: casting DMA straight to fp8e4 (1 B/weight, no engine work; the
         TRN fp8 rounding on ~1/3 of the contraction adds ~1e-2 rms,
         within the 2e-2 gate - verified in sim + HW)
      S: casting DMA to bf16 (2 B/weight, no engine work)
      D/A: raw int8 + DVE (~1.2us) / ACT (~2.0us) dequant to bf16
  - PE uses 4-way column tiling (tile_position=(0, 32j)): four concurrent
    [128k, 16m] x [128k, 512o] matmul streams accumulate into one PSUM bank
  - x*scale is bf16 (hi only)
  - bias folds in as a final accumulation matmul against a one-hot lhsT;
    the epilogue is PSUM->SBUF copies on DVE and ACT in parallel, each
    chased by its own output DMA
"""

import os

import numpy as np

IN_F = 4096
OUT_F = 16384
NT = 16
NCORES = 8
O_PER = OUT_F // NCORES  # 2048
NCH = IN_F // 128  # 32 k-chunks
NTJ = 4  # PE column tiles
NMM = 512  # moving free size per matmul

# Per-chunk source assignment (see module docstring). F heads the stream
# (PE starts without engine casts) and tails it (no cast backlog at the
# end); S relieves the engines mid-stream.
ASSIGN = [
    "F", "F", "D", "A", "D", "S", "D", "A", "F", "D", "D", "A", "D", "S", "D", "A",
    "F", "F", "D", "A", "D", "S", "D", "A", "F", "D", "D", "S", "D", "D", "F", "F",
]
assert len(ASSIGN) == NCH

_CACHE = {}
LAST_EXEC_NS = None


def _install_drain_patch():
    """walrus codegen only allows 1 sem-wait per SP instruction; Tile's
    kernel-tail Drain aggregates many. Split them across sync nops."""
    from concourse.tile import TileContext
    from concourse.tile_scheduler import N_PROCS
    from concourse.vector_clock import VectorClock
    from bass_rust import ScopedClock

    if getattr(TileContext, "_drain_patched", False):
        return

    def _patched(self, tick_clock, wait_clock):
        gc = tick_clock.global_clock
        ticks = [gc[p] for p in range(N_PROCS)]
        for i in range(N_PROCS):
            partial = VectorClock(
                [ticks[p] if p == i else 0 for p in range(N_PROCS)]
            )
            if all(t == 0 for t in partial):
                continue
            nop = self.nc.sync.nop(hint="tail_wait", nofuse=True)
            wait_clock.add_sem_waits(nop.ins, ScopedClock({None: partial}))
        self.nc.sync.drain()
        self.nc.all_engine_barrier()
        assert self.sems is not None
        popped = self.nc._tile_sem_poison_stack.pop()
        assert popped is self._sem_poison
        self.nc.clear_and_free_semaphores(list(self.sems.allocated().values()))
        self.nc.all_engine_barrier()

    TileContext._drain_and_barrier = _patched
    TileContext._drain_patched = True


def _split_multi_waits(nc):
    """walrus codegen allows only one sem-wait per instruction: hoist all
    but the last wait of any instruction onto same-engine NoOps before it."""
    from concourse import mybir

    cnt = 0
    for fn in nc.m.functions:
        for bb in fn.blocks:
            out = []
            for inst in bb.instructions:
                si = inst.sync_info
                if si is not None and si.on_wait and len(si.on_wait) > 1:
                    waits = list(si.on_wait)
                    for w in waits[:-1]:
                        cnt += 1
                        nop = mybir.InstNoOp(
                            name=f"{inst.name}-sw{cnt}", ins=[], outs=[]
                        )
                        nop.engine = inst.engine
                        nop.sync_info = mybir.SyncInfo(on_wait=[w], on_update=[])
                        out.append(nop)
                    si.on_wait = [waits[-1]]
                out.append(inst)
            bb.instructions[:] = out


def _build_nc(for_hw=True):
    import concourse.bass as bass
    import concourse.mybir as mybir
    from concourse.tile import TileContext

    _install_drain_patch()

    nc = bass.Bass(trn_type="TRN2")
    # xt cols [16n, 16n+16) = k-chunk n of (x*scale)^T in bf16;
    # cols [512:528) = one-hot block (partition 0 = 1.0) for the bias matmul
    xt = nc.dram_tensor("xt", [128, NCH * NT + NT], mybir.dt.bfloat16, kind="ExternalInput")
    # wt col = n*2048 + j*512 + c  =  W[o = j*512 + c, k = n*128 + p]
    wt = nc.dram_tensor("wt", [128, IN_F * O_PER // 128], mybir.dt.int8, kind="ExternalInput")
    bs = nc.dram_tensor("bs", [1, O_PER], mybir.dt.bfloat16, kind="ExternalInput")
    y = nc.dram_tensor("y", [NT, O_PER], mybir.dt.float32, kind="ExternalOutput")

    with TileContext(nc) as tc:
        with (
            tc.tile_pool(name="xp", bufs=1) as xp,
            tc.tile_pool(name="bp", bufs=1) as bp,
            tc.tile_pool(name="w8", bufs=6) as w8p,
            tc.tile_pool(name="wb", bufs=12) as wbp,
            tc.tile_pool(name="ws", bufs=4) as wsp,
            tc.tile_pool(name="wf", bufs=4) as wfp,
            tc.tile_pool(name="pp", bufs=1, space="PSUM") as pp,
            tc.tile_pool(name="op", bufs=1) as op,
        ):
            # xt rides the fast SWDGE queue ahead of the weight stream;
            # the bias plane (row 0 = bias bf16, rest zeroed by ACT while it
            # is otherwise idle) feeds the tail bias-accumulation matmul
            xsb = xp.tile([128, NCH * NT + NT], mybir.dt.bfloat16)
            nc.gpsimd.dma_start(out=xsb[:], in_=xt[:])
            bsb = bp.tile([128, O_PER], mybir.dt.bfloat16)
            nc.scalar.memzero(bsb[:])
            nc.sync.dma_start(out=bsb[0:1, :], in_=bs[:])
            ones = xsb[:, NCH * NT : NCH * NT + NT]  # [128, 16] one-hot

            psum = pp.tile([128, NMM], mybir.dt.float32, tag="ps", name="ps")

            # PE warmup: the HAM clock gate only lifts to 2.4 GHz after
            # ~3.4us of sustained matmul activity; burn that in on a scratch
            # PSUM bank while the DMA pipe fills, so the real stream runs
            # warm (cold N=512 matmuls measured 739ns vs ~230ns warm)
            wup = pp.tile([128, NMM], mybir.dt.float32, tag="wu", name="wu")
            for wi in range(14):
                nc.tensor.matmul(
                    wup[0:NT, :],
                    lhsT=ones,
                    rhs=xsb[:, 0:NMM],
                    start=(wi == 0),
                    stop=False,
                    tile_position=(0, 0),
                    skip_group_check=True,
                )

            def keepalive(stop=False):
                # dependency-free matmul on the scratch bank: runs the
                # moment the PE queue reaches it, so stream hiccups can't
                # leave the PE idle for a full HAM window (re-throttle)
                nc.tensor.matmul(
                    wup[0:NT, :],
                    lhsT=ones,
                    rhs=xsb[:, 0:NMM],
                    start=False,
                    stop=stop,
                    tile_position=(0, 0),
                    skip_group_check=True,
                )

            # plan DMA ops: adjacent same-kind chunks merge into [128, 4096]
            units = []  # (kind, [n...])
            n = 0
            while n < NCH:
                kind = ASSIGN[n] if ASSIGN[n] in ("S", "F") else "R"
                nkind = (
                    (ASSIGN[n + 1] if ASSIGN[n + 1] in ("S", "F") else "R")
                    if n + 1 < NCH
                    else None
                )
                if kind in ("R", "F") and nkind == kind and n != 2:
                    units.append((kind, [n, n + 1]))
                    n += 2
                else:
                    units.append((kind, [n]))
                    n += 1

            wsrc = {}
            for kind, ns in units:
                base = ns[0] * 2048
                span = 2048 * len(ns)
                if kind == "F":
                    tag = "wf" if len(ns) == 2 else "wf1"
                    wb = wfp.tile(
                        [128, span], mybir.dt.float8e4, tag=tag, name=f"wf_{ns[0]}"
                    )
                    nc.gpsimd.dma_start(out=wb[:], in_=wt[:, base : base + span])
                    for k, nn in enumerate(ns):
                        wsrc[nn] = (wb, k * 2048)
                elif kind == "S":
                    wb = wsp.tile(
                        [128, span], mybir.dt.bfloat16, tag="wbs", name=f"ws_{ns[0]}"
                    )
                    nc.gpsimd.dma_start(out=wb[:], in_=wt[:, base : base + span])
                    wsrc[ns[0]] = (wb, 0)
                else:
                    tag = "w8" if len(ns) == 2 else "w81"
                    w8 = w8p.tile(
                        [128, span], mybir.dt.int8, tag=tag, name=f"w8_{ns[0]}"
                    )
                    if ns[0] == 2:
                        # prime the cast engines ~2us early via the
                        # otherwise-idle HWDGE ring
                        nc.sync.dma_start(out=w8[:], in_=wt[:, base : base + span])
                    else:
                        nc.gpsimd.dma_start(out=w8[:], in_=wt[:, base : base + span])
                    for k, nn in enumerate(ns):
                        wb = wbp.tile(
                            [128, 2048], mybir.dt.bfloat16, tag="wb", name=f"wb_{nn}"
                        )
                        src = w8[:, k * 2048 : (k + 1) * 2048]
                        if ASSIGN[nn] == "A":
                            nc.scalar.copy(wb[:], src)
                        else:
                            nc.vector.tensor_copy(wb[:], src)
                        wsrc[nn] = (wb, 0)

            # matmul stream: 4 concurrent column-tile chains over 32 chunks
            for n in range(NCH):
                wb, off = wsrc[n]
                for j in range(NTJ):
                    nc.tensor.matmul(
                        psum[32 * j : 32 * j + NT, :],
                        lhsT=xsb[:, NT * n : NT * (n + 1)],
                        rhs=wb[:, off + j * 512 : off + (j + 1) * 512],
                        start=(n == 0),
                        stop=False,
                        tile_position=(0, 32 * j),
                        skip_group_check=True,
                    )
                if n % 2 == 1:
                    keepalive(stop=(n == NCH - 1))
            # bias folds in as the final accumulation against a one-hot
            # lhsT; per column group: bias matmul -> copy -> output DMA so
            # the four groups pipeline across PE, DVE/ACT and the DMA ring
            osb = op.tile([NT, O_PER], mybir.dt.float32, tag="o")
            for j in range(NTJ):
                nc.tensor.matmul(
                    psum[32 * j : 32 * j + NT, :],
                    lhsT=ones,
                    rhs=bsb[:, j * 512 : (j + 1) * 512],
                    start=False,
                    stop=True,
                    tile_position=(0, 32 * j),
                    skip_group_check=True,
                )
            for j in range(NTJ):
                dst = osb[:, j * 512 : (j + 1) * 512]
                srcp = psum[32 * j : 32 * j + NT, :]
                if j % 2 == 0:
                    nc.vector.tensor_copy(dst, srcp)
                else:
                    nc.scalar.copy(dst, srcp)
                nc.sync.dma_start(
                    out=y[:, j * 512 : (j + 1) * 512],
                    in_=dst,
                )
    if for_hw:
        _split_multi_waits(nc)
    return nc


def _prep(x, w, scale, bias):
    import ml_dtypes

    xs = (np.asarray(x, dtype=np.float32) * np.float32(scale)).astype(
        ml_dtypes.bfloat16
    )
    # xt[p, 16n + t] = xs[t, n*128 + p]; one-hot block for the bias matmul
    xt_host = np.zeros((128, NCH * NT + NT), dtype=ml_dtypes.bfloat16)
    xt_host[:, : NCH * NT] = (
        xs.T.reshape(NCH, 128, NT).transpose(1, 0, 2).reshape(128, NCH * NT)
    )
    xt_host[0, NCH * NT :] = ml_dtypes.bfloat16(1.0)
    return xt_host


def _prep_w(wshard):
    # wt2[p, n*2048 + j*512 + c] = wshard[j*512 + c, n*128 + p]
    A = np.ascontiguousarray(wshard.T)  # [4096 k, 2048 o]
    A4 = A.reshape(NCH, 128, O_PER)  # (n, p, o)
    return np.ascontiguousarray(
        A4.transpose(1, 0, 2).reshape(128, IN_F * O_PER // 128)
    )


def kernel(x, weight_int8, weight_scale, bias):
    global LAST_EXEC_NS
    import ml_dtypes
    from concourse.bass_utils import run_bass_kernel_spmd

    w = np.asarray(weight_int8)
    if w.dtype != np.int8:
        w = w.astype(np.int8)
    scale = float(np.asarray(weight_scale, dtype=np.float32))
    bias = np.asarray(bias, dtype=np.float32)

    xt_host = _prep(x, w, scale, bias)

    if "nc" not in _CACHE:
        _CACHE["nc"] = _build_nc()
    nc = _CACHE["nc"]

    in_maps = []
    for c in range(NCORES):
        wt_c = _prep_w(w[c * O_PER : (c + 1) * O_PER, :])
        bs_c = np.ascontiguousarray(
            bias[c * O_PER : (c + 1) * O_PER].astype(ml_dtypes.bfloat16)[None, :]
        )
        in_maps.append({"xt": xt_host, "wt": wt_c, "bs": bs_c})

    trace = bool(os.environ.get("BASS_KERNEL_TRACE"))
    br = run_bass_kernel_spmd(nc, in_maps, list(range(NCORES)), trace=trace)
    LAST_EXEC_NS = br.exec_time_ns
    return np.concatenate([br.results[c]["y"] for c in range(NCORES)], axis=1)


# revision 34
# speedup vs baseline: 1.0677x; 1.0245x over previous
"""Int8Linear TRN2 kernel: y = x @ (W_int8 * scale)^T + bias.  (v7)

Column-parallel across 8 NeuronCores: each core owns a [2048, 4096] shard
of W, the full x, and its bias slice.

  - ALL weight traffic rides SWDGE (gpsimd DMA, ~410 GB/s measured) except
    one early chunk primed on the otherwise-idle HWDGE ring
  - per chunk-pair unit: F = casting DMA to fp8e4, S = casting DMA to
    bf16, D/A = raw int8 + DVE/ACT dequant
  - PE uses 4-way column tiling; o-space split in two halves with separate
    PSUM banks; bias folds in as a tail accumulation matmul; epilogue is
    parallel DVE/ACT PSUM->SBUF copies chased by per-group output DMAs
"""

import os

import numpy as np

IN_F = 4096
OUT_F = 16384
NT = 16
NCORES = 8
O_PER = OUT_F // NCORES  # 2048
NCH = IN_F // 128  # 32 k-chunks
NHALF = 2  # o halves of 1024
NPAIR = NCH // 2  # 16 chunk-pairs (cast units) per half
NTJ = 4  # PE column tiles
NMM = 256  # moving free size per matmul

ASSIGN = (
    ["F", "F", "D", "A", "D", "D", "A", "D", "D", "D", "A", "D", "D", "D", "F", "F"]
    + ["F", "F", "D", "A", "D", "D", "A", "D", "S", "A", "D", "D", "A", "D", "F", "F"]
)
assert len(ASSIGN) == NHALF * NPAIR

_CACHE = {}
LAST_EXEC_NS = None


def _install_drain_patch():
    """walrus codegen only allows 1 sem-wait per SP instruction; Tile's
    kernel-tail Drain aggregates many. Split them across sync nops."""
    from concourse.tile import TileContext
    from concourse.tile_scheduler import N_PROCS
    from concourse.vector_clock import VectorClock
    from bass_rust import ScopedClock

    if getattr(TileContext, "_drain_patched", False):
        return

    def _patched(self, tick_clock, wait_clock):
        gc = tick_clock.global_clock
        ticks = [gc[p] for p in range(N_PROCS)]
        for i in range(N_PROCS):
            partial = VectorClock(
                [ticks[p] if p == i else 0 for p in range(N_PROCS)]
            )
            if all(t == 0 for t in partial):
                continue
            nop = self.nc.sync.nop(hint="tail_wait", nofuse=True)
            wait_clock.add_sem_waits(nop.ins, ScopedClock({None: partial}))
        self.nc.sync.drain()
        self.nc.all_engine_barrier()
        assert self.sems is not None
        popped = self.nc._tile_sem_poison_stack.pop()
        assert popped is self._sem_poison
        self.nc.clear_and_free_semaphores(list(self.sems.allocated().values()))
        self.nc.all_engine_barrier()

    TileContext._drain_and_barrier = _patched
    TileContext._drain_patched = True


def _split_multi_waits(nc):
    """walrus codegen allows only one sem-wait per instruction: hoist all
    but the last wait of any instruction onto same-engine NoOps before it."""
    from concourse import mybir

    cnt = 0
    for fn in nc.m.functions:
        for bb in fn.blocks:
            out = []
            for inst in bb.instructions:
                si = inst.sync_info
                if si is not None and si.on_wait and len(si.on_wait) > 1:
                    waits = list(si.on_wait)
                    for w in waits[:-1]:
                        cnt += 1
                        nop = mybir.InstNoOp(
                            name=f"{inst.name}-sw{cnt}", ins=[], outs=[]
                        )
                        nop.engine = inst.engine
                        nop.sync_info = mybir.SyncInfo(on_wait=[w], on_update=[])
                        out.append(nop)
                    si.on_wait = [waits[-1]]
                out.append(inst)
            bb.instructions[:] = out


def _build_nc(for_hw=True):
    import concourse.bass as bass
    import concourse.mybir as mybir
    from concourse.tile import TileContext

    _install_drain_patch()

    nc = bass.Bass(trn_type="TRN2")
    xt = nc.dram_tensor("xt", [128, NCH * NT + NT], mybir.dt.bfloat16, kind="ExternalInput")
    # wt cols: h*32768 + q*2048 + i*1024 + j*256 + c
    #   = W[o = h*1024 + j*256 + c, k = (2q+i)*128 + p]
    wt = nc.dram_tensor("wt", [128, IN_F * O_PER // 128], mybir.dt.int8, kind="ExternalInput")
    bs = nc.dram_tensor("bs", [1, O_PER], mybir.dt.bfloat16, kind="ExternalInput")
    y = nc.dram_tensor("y", [NT, O_PER], mybir.dt.float32, kind="ExternalOutput")

    with TileContext(nc) as tc:
        with (
            tc.tile_pool(name="xp", bufs=1) as xp,
            tc.tile_pool(name="bp", bufs=1) as bp,
            tc.tile_pool(name="w8", bufs=8) as w8p,
            tc.tile_pool(name="wb", bufs=12) as wbp,
            tc.tile_pool(name="ws", bufs=2) as wsp,
            tc.tile_pool(name="wf", bufs=2) as wfp,
            tc.tile_pool(name="pp", bufs=1, space="PSUM") as pp,
            tc.tile_pool(name="op", bufs=2) as op,
        ):
            xsb = xp.tile([128, NCH * NT + NT], mybir.dt.bfloat16)
            nc.gpsimd.dma_start(out=xsb[:], in_=xt[:])
            bsb = bp.tile([128, O_PER], mybir.dt.bfloat16)
            nc.scalar.memzero(bsb[:])
            nc.sync.dma_start(out=bsb[0:1, :], in_=bs[:])
            ones = xsb[:, NCH * NT : NCH * NT + NT]  # [128, 16] one-hot

            psums = [
                pp.tile([128, NMM], mybir.dt.float32, tag=f"ps{h}", name=f"ps{h}")
                for h in range(NHALF)
            ]

            for h in range(NHALF):
                units = []  # (kind, [u...])
                q = 0
                while q < NPAIR:
                    u = h * NPAIR + q
                    kind = ASSIGN[u] if ASSIGN[u] in ("S", "F") else "R"
                    nkind = (
                        (ASSIGN[u + 1] if ASSIGN[u + 1] in ("S", "F") else "R")
                        if q + 1 < NPAIR
                        else None
                    )
                    if kind in ("R", "F") and nkind == kind and u != 2:
                        units.append((kind, [u, u + 1]))
                        q += 2
                    else:
                        units.append((kind, [u]))
                        q += 1

                wsrc = {}
                for kind, us in units:
                    q0 = us[0] % NPAIR
                    base = h * 32768 + q0 * 2048
                    span = 2048 * len(us)
                    if kind == "F":
                        tag = "wf" if len(us) == 2 else "wf1"
                        wb = wfp.tile(
                            [128, span], mybir.dt.float8e4, tag=tag, name=f"wf_{us[0]}"
                        )
                        nc.gpsimd.dma_start(out=wb[:], in_=wt[:, base : base + span])
                        for k, uu in enumerate(us):
                            wsrc[uu] = (wb, k * 2048)
                    elif kind == "S":
                        wb = wsp.tile(
                            [128, span], mybir.dt.bfloat16, tag="wbs", name=f"ws_{us[0]}"
                        )
                        nc.gpsimd.dma_start(out=wb[:], in_=wt[:, base : base + span])
                        wsrc[us[0]] = (wb, 0)
                    else:
                        tag = "w8" if len(us) == 2 else "w81"
                        w8 = w8p.tile(
                            [128, span], mybir.dt.int8, tag=tag, name=f"w8_{us[0]}"
                        )
                        if us[0] == 2:
                            nc.sync.dma_start(out=w8[:], in_=wt[:, base : base + span])
                        else:
                            nc.gpsimd.dma_start(out=w8[:], in_=wt[:, base : base + span])
                        for k, uu in enumerate(us):
                            wb = wbp.tile(
                                [128, 2048], mybir.dt.bfloat16, tag="wb",
                                name=f"wb_{uu}",
                            )
                            src = w8[:, k * 2048 : (k + 1) * 2048]
                            if ASSIGN[uu] == "A":
                                nc.scalar.copy(wb[:], src)
                            else:
                                nc.vector.tensor_copy(wb[:], src)
                            wsrc[uu] = (wb, 0)

                for q in range(NPAIR):
                    uu = h * NPAIR + q
                    wb, off = wsrc[uu]
                    for i in range(2):
                        n = 2 * q + i
                        first = q == 0 and i == 0
                        for j in range(NTJ):
                            nc.tensor.matmul(
                                psums[h][32 * j : 32 * j + NT, :],
                                lhsT=xsb[:, NT * n : NT * (n + 1)],
                                rhs=wb[
                                    :,
                                    off + i * 1024 + j * 256 : off + i * 1024 + (j + 1) * 256,
                                ],
                                start=first,
                                stop=False,
                                tile_position=(0, 32 * j),
                                skip_group_check=True,
                            )
                for j in range(NTJ):
                    nc.tensor.matmul(
                        psums[h][32 * j : 32 * j + NT, :],
                        lhsT=ones,
                        rhs=bsb[:, h * 1024 + j * 256 : h * 1024 + (j + 1) * 256],
                        start=False,
                        stop=True,
                        tile_position=(0, 32 * j),
                        skip_group_check=True,
                    )
                osb = op.tile([NT, 1024], mybir.dt.float32, tag=f"o{h}")
                for j in (0, 2, 1, 3):
                    dst = osb[:, j * 256 : (j + 1) * 256]
                    srcp = psums[h][32 * j : 32 * j + NT, :]
                    if j % 2 == 0:
                        nc.vector.tensor_copy(dst, srcp)
                    else:
                        nc.scalar.copy(dst, srcp)
                    nc.sync.dma_start(
                        out=y[:, h * 1024 + j * 256 : h * 1024 + (j + 1) * 256],
                        in_=dst,
                    )
    if for_hw:
        _split_multi_waits(nc)
    return nc


def _prep(x, w, scale, bias):
    import ml_dtypes

    xs = (np.asarray(x, dtype=np.float32) * np.float32(scale)).astype(
        ml_dtypes.bfloat16
    )
    # xt[p, 16n + t] = xs[t, n*128 + p]; one-hot block for the bias matmul
    xt_host = np.zeros((128, NCH * NT + NT), dtype=ml_dtypes.bfloat16)
    xt_host[:, : NCH * NT] = (
        xs.T.reshape(NCH, 128, NT).transpose(1, 0, 2).reshape(128, NCH * NT)
    )
    xt_host[0, NCH * NT :] = ml_dtypes.bfloat16(1.0)
    return xt_host


def _prep_w(wshard):
    # wt2[p, h*32768 + q*2048 + i*1024 + j*256 + c]
    #   = wshard[h*1024 + j*256 + c, (2q+i)*128 + p]
    A = np.ascontiguousarray(wshard.T)  # [4096 k, 2048 o]
    A6 = A.reshape(NPAIR, 2, 128, NHALF, NTJ, NMM)  # (q, i, p, h, j, c)
    return np.ascontiguousarray(
        A6.transpose(2, 3, 0, 1, 4, 5).reshape(128, IN_F * O_PER // 128)
    )


def kernel(x, weight_int8, weight_scale, bias):
    global LAST_EXEC_NS
    import ml_dtypes
    from concourse.bass_utils import run_bass_kernel_spmd

    w = np.asarray(weight_int8)
    if w.dtype != np.int8:
        w = w.astype(np.int8)
    scale = float(np.asarray(weight_scale, dtype=np.float32))
    bias = np.asarray(bias, dtype=np.float32)

    xt_host = _prep(x, w, scale, bias)

    if "nc" not in _CACHE:
        _CACHE["nc"] = _build_nc()
    nc = _CACHE["nc"]

    in_maps = []
    for c in range(NCORES):
        wt_c = _prep_w(w[c * O_PER : (c + 1) * O_PER, :])
        bs_c = np.ascontiguousarray(
            bias[c * O_PER : (c + 1) * O_PER].astype(ml_dtypes.bfloat16)[None, :]
        )
        in_maps.append({"xt": xt_host, "wt": wt_c, "bs": bs_c})

    trace = bool(os.environ.get("BASS_KERNEL_TRACE"))
    br = run_bass_kernel_spmd(nc, in_maps, list(range(NCORES)), trace=trace)
    LAST_EXEC_NS = br.exec_time_ns
    return np.concatenate([br.results[c]["y"] for c in range(NCORES)], axis=1)
